# revision 12
# baseline (speedup 1.0000x reference)
"""Trainium2 Bass kernel for nn_DictionaryLearningBottleneck.

Computes batched OMP (5 iterations, 512-atom dictionary, 64-dim signals)
plus straight-through-quantized output and losses, data-parallel over the
signal axis across 8 NeuronCores.

Self-contained: hardcodes shapes; host-side prep is limited to layout
transforms (transposes/reshapes of the dictionary), output unsharding,
the final scalar loss reduction, and densifying the sparse coefficients.
"""
import numpy as np

B, E, HWF, K, S = 32, 64, 1024, 512, 5
N_CORES = 8
BPC = B // N_CORES            # batch slices per core
NSIG = BPC * HWF              # 4096 signals per core
T = NSIG // 128               # 32 tiles of 128 signals
NQ = 4                        # gather DMA queues (ucode MAX_SWDGE_QUEUES)
TQ = T // NQ                  # tiles per queue
EPS = 1e-10
NEGBIG = -1.0e30
COMMITMENT_COST = 0.25
COEFF_CLAMP = 1e7

_CACHE = {}
TRACE = False
LAST_EXEC_NS = None
LAST_RESULT = None


def _install_profhook():
    """Enable NTFF profiling through the axon tunnel (trace=True path)."""
    import sys, types
    if "antenv.axon_hooks" in sys.modules:
        return
    try:
        import antenv
        from trn_agent_boot.trn_boot import _ntff_profile_via_ctypes
        hook = _ntff_profile_via_ctypes("/opt/axon/libaxon_pjrt.so")
        mod = types.ModuleType("antenv.axon_hooks")
        mod.get_axon_ntff_profile_hook = lambda: hook
        mod.set_axon_ntff_profile_hook = lambda h: None
        sys.modules["antenv.axon_hooks"] = mod
        antenv.axon_hooks = mod
    except Exception:
        pass


def _build(nsig=NSIG, bpc=BPC, hwf=None):
    import concourse.tile as tile
    from concourse import bacc, mybir

    f32 = mybir.dt.float32
    i32 = mybir.dt.int32
    i16 = mybir.dt.int16
    Alu = mybir.AluOpType
    Act = mybir.ActivationFunctionType
    X = mybir.AxisListType.X

    t_n = nsig // 128
    tq = t_n // NQ
    if hwf is None:
        hwf = nsig // bpc

    nc = bacc.Bacc("TRN2", target_bir_lowering=False, debug=False,
                   num_devices=N_CORES, num_swdge_queues=NQ)

    zs = nc.dram_tensor("zs", [bpc, E, hwf], f32, kind="ExternalInput").ap()
    dmat = nc.dram_tensor("dmat", [E, K], f32, kind="ExternalInput").ap()
    dtr = nc.dram_tensor("dtr", [K, E], f32, kind="ExternalInput").ap()
    o_zq = nc.dram_tensor("o_zq", [bpc, E, hwf], f32, kind="ExternalOutput").ap()
    o_idx = nc.dram_tensor("o_idx", [t_n, 128, S], f32, kind="ExternalOutput").ap()
    o_c = nc.dram_tensor("o_c", [t_n, 128, S], f32, kind="ExternalOutput").ap()
    o_rec = nc.dram_tensor("o_rec", [E, 1], f32, kind="ExternalOutput").ap()
    o_l1 = nc.dram_tensor("o_l1", [128, 1], f32, kind="ExternalOutput").ap()

    with tile.TileContext(nc) as tc:
        with tc.tile_pool(name="st", bufs=1) as st, \
             tc.tile_pool(name="dram", bufs=1, space="DRAM") as dpool, \
             tc.tile_pool(name="pcorr", bufs=3, space="PSUM") as pcorr, \
             tc.tile_pool(name="pmu", bufs=3, space="PSUM") as pmu, \
             tc.tile_pool(name="scr", bufs=3) as scr, \
             tc.tile_pool(name="big", bufs=1) as big:

            # ---------------- static tiles ----------------
            s_em = st.tile([E, nsig], f32, tag="s_em")
            for b in range(bpc):
                eng = [nc.sync, nc.scalar][b % 2]
                eng.dma_start(
                    s_em.rearrange("e (b h) -> e b h", b=bpc)[:, b, :], zs[b])

            dD = st.tile([E, K], f32, tag="dD")
            nc.sync.dma_start(dD[:], dmat[:])

            iota_i = st.tile([128, K], i32, tag="iota_i")
            nc.gpsimd.iota(iota_i[:], pattern=[[1, K]], base=0,
                           channel_multiplier=0)
            iota_f = st.tile([128, K], f32, tag="iota_f")
            nc.vector.tensor_copy(iota_f[:], iota_i[:])

            iota1_i = st.tile([128, K], i32, tag="iota1_i")
            nc.gpsimd.iota(iota1_i[:], pattern=[[1, K]], base=1,
                           channel_multiplier=0)
            iota1_f = st.tile([128, K], f32, tag="iota1_f")
            nc.vector.tensor_copy(iota1_f[:], iota1_i[:])

            iota_p = st.tile([128, 1], i32, tag="iota_p")
            nc.gpsimd.iota(iota_p[:], pattern=[[0, 1]], base=0,
                           channel_multiplier=1)
            iota_pf = st.tile([128, 1], f32, tag="iota_pf")
            nc.vector.tensor_copy(iota_pf[:], iota_p[:])
            ones128 = st.tile([128, 128], f32, tag="ones128")
            nc.vector.memset(ones128[:], 1.0)
            ident = st.tile([128, 128], f32, tag="ident")
            nc.vector.scalar_tensor_tensor(
                out=ident[:], in0=iota_f[:, 0:128], scalar=iota_pf[:],
                in1=ones128[:], op0=Alu.is_equal, op1=Alu.mult)

            maskst = st.tile([128, t_n, K], mybir.dt.bfloat16, tag="maskst")
            nc.vector.memset(maskst[:], 0.0)
            a_st = st.tile([128, t_n, S, E], f32, tag="a_st")
            mu_em = st.tile([E, nsig], f32, tag="mu_em")
            nc.vector.memset(mu_em[:], 0.0)

            gmat = st.tile([128, t_n, S, S], f32, tag="gmat")
            minv = st.tile([128, t_n, S, S], f32, tag="minv")
            cst = st.tile([128, t_n, S], f32, tag="cst")
            nc.vector.memset(cst[:], 0.0)
            mst = st.tile([128, t_n, S], f32, tag="mst")
            c0a = st.tile([128, t_n, S], f32, tag="c0a")
            idxf = st.tile([128, t_n, S], f32, tag="idxf")
            y5 = st.tile([128, t_n, S], f32, tag="y5")
            v5 = st.tile([128, t_n, S], f32, tag="v5")
            tkk = st.tile([128, t_n], f32, tag="tkk")
            tss = st.tile([128, t_n], f32, tag="tss")
            sv = st.tile([128, t_n], f32, tag="sv")
            tk5 = st.tile([128, t_n, S], f32, tag="tk5")
            vv = st.tile([128, t_n, S, S], f32, tag="vv")
            negc = st.tile([128, t_n, S], f32, tag="negc")
            idx32 = st.tile([128, t_n], i32, tag="idx32")
            gidx16 = st.tile([128, t_n], i16, tag="gidx16")
            wrapa = st.tile([128, t_n * 8], i16, tag="wrapa")

            bufa = dpool.tile([t_n, 128], i16, tag="bufa")

            agq = [st.tile([128, tq, E], f32, name=f"agq{q}", tag=f"agq{q}")
                   for q in range(NQ)]

            # ---------------- OMP iterations ----------------
            for k in range(S):
                # --- selection phase (per tile) ---
                for t in range(t_n):
                    ps = pcorr.tile([128, K], f32, tag="ps")
                    nc.tensor.matmul(ps[:], s_em[:, t * 128:(t + 1) * 128],
                                     dD[:], start=True, stop=(k == 0))
                    if k > 0:
                        nc.tensor.matmul(ps[:], mu_em[:, t * 128:(t + 1) * 128],
                                         dD[:], start=False, stop=True)
                    corrm = scr.tile([128, K], f32, tag="corrm")
                    nc.vector.scalar_tensor_tensor(
                        out=corrm[:], in0=maskst[:, t, :], scalar=NEGBIG,
                        in1=ps[:], op0=Alu.mult, op1=Alu.add)
                    m8 = scr.tile([128, 8], f32, tag="m8")
                    nc.vector.max(out=m8[:], in_=corrm[:])
                    nc.scalar.activation(mst[:, t, k:k + 1], m8[:, 0:1],
                                         Act.Copy)
                    tr = scr.tile([128, K], mybir.dt.bfloat16, tag="tr")
                    nc.vector.scalar_tensor_tensor(
                        out=tr[:], in0=corrm[:], scalar=m8[:, 0:1],
                        in1=iota1_f[:], op0=Alu.is_ge, op1=Alu.mult,
                        accum_out=idxf[:, t, k:k + 1])
                    if k < S - 1:
                        nc.gpsimd.tensor_add(maskst[:, t, :],
                                             maskst[:, t, :], tr[:])

                # --- gather new atom rows ---
                nc.vector.tensor_scalar(idxf[:, :, k], idxf[:, :, k], 1.0,
                                        scalar2=None, op0=Alu.subtract)
                nc.vector.tensor_copy(idx32[:], idxf[:, :, k])
                nc.vector.tensor_copy(gidx16[:], idx32[:])
                nc.sync.dma_start(bufa.rearrange("t p -> p t"), gidx16[:])
                wr_in = bufa.rearrange("t p -> (t p)") \
                            .rearrange("(jj pl) -> pl jj", pl=16)
                for g in range(8):
                    eng = [nc.scalar, nc.sync][g % 2]
                    eng.dma_start(wrapa[g * 16:(g + 1) * 16, :], wr_in)
                for q in range(NQ):
                    nc.gpsimd.dma_gather(
                        out_ap=agq[q][:], in_ap=dtr[:],
                        idxs_ap=wrapa[:, q * tq * 8:(q + 1) * tq * 8],
                        num_idxs=tq * 128, num_idxs_reg=tq * 128,
                        elem_size=E, queue_num=q)
                for q in range(NQ):
                    nc.scalar.activation(a_st[:, q * tq:(q + 1) * tq, k, :],
                                         agq[q][:], Act.Copy)

                # --- Gram column via dots: G[i,k] = <A_i, A_k>, i<=k ---
                th = t_n // 2
                for h in range(2):
                    hs = slice(h * th, (h + 1) * th)
                    ab = a_st[:, hs, k, :][:, :, None, :].to_broadcast(
                        [128, th, k + 1, E])
                    gp = big.tile([128, th, S, E], f32, tag="gp")
                    nc.vector.tensor_mul(gp[:, :, 0:k + 1, :],
                                         a_st[:, hs, 0:k + 1, :], ab)
                    nc.vector.tensor_reduce(out=gmat[:, hs, 0:k + 1, k],
                                            in_=gp[:, :, 0:k + 1, :],
                                            axis=X, op=Alu.add)
                if k > 0:
                    nc.gpsimd.tensor_copy(gmat[:, :, k, 0:k],
                                          gmat[:, :, 0:k, k])

                # --- solve (G + eps I) c = y via block-inverse recursion ---
                if k == 0:
                    nc.vector.tensor_copy(c0a[:, :, 0], mst[:, :, 0])
                    nc.vector.reciprocal(minv[:, :, 0, 0], gmat[:, :, 0, 0])
                    nc.vector.tensor_mul(cst[:, :, 0], minv[:, :, 0, 0],
                                         mst[:, :, 0])
                else:
                    # corr0 at the new atom: m_k + sum_{j<k} G[k,j] c_j
                    nc.gpsimd.tensor_mul(tk5[:, :, 0:k], gmat[:, :, k, 0:k],
                                         cst[:, :, 0:k])
                    nc.vector.tensor_reduce(out=tkk[:], in_=tk5[:, :, 0:k],
                                            axis=X, op=Alu.add)
                    nc.gpsimd.tensor_add(c0a[:, :, k], mst[:, :, k], tkk[:])
                    # y_i = corr0a_i - sum_{j<k} G[i,j] c_j   (i <= k)
                    cb = cst[:, :, 0:k][:, :, None, :].to_broadcast(
                        [128, t_n, k + 1, k])
                    nc.vector.tensor_mul(vv[:, :, 0:k + 1, 0:k],
                                         gmat[:, :, 0:k + 1, 0:k], cb)
                    nc.vector.tensor_reduce(out=y5[:, :, 0:k + 1],
                                            in_=vv[:, :, 0:k + 1, 0:k],
                                            axis=X, op=Alu.add)
                    nc.vector.tensor_sub(y5[:, :, 0:k + 1],
                                         c0a[:, :, 0:k + 1],
                                         y5[:, :, 0:k + 1])
                    # v = Minv b ; ss = G[k,k] - b.v ; sinv = 1/ss
                    bb = gmat[:, :, 0:k, k][:, :, None, :].to_broadcast(
                        [128, t_n, k, k])
                    nc.vector.tensor_mul(vv[:, :, 0:k, 0:k],
                                         minv[:, :, 0:k, 0:k], bb)
                    nc.vector.tensor_reduce(out=v5[:, :, 0:k],
                                            in_=vv[:, :, 0:k, 0:k],
                                            axis=X, op=Alu.add)
                    nc.gpsimd.tensor_mul(tk5[:, :, 0:k], gmat[:, :, 0:k, k],
                                         v5[:, :, 0:k])
                    nc.vector.tensor_reduce(out=tkk[:], in_=tk5[:, :, 0:k],
                                            axis=X, op=Alu.add)
                    nc.gpsimd.tensor_sub(tss[:], gmat[:, :, k, k], tkk[:])
                    nc.vector.reciprocal(sv[:], tss[:])
                    # Minv[0:k,0:k] += sinv * v v^T
                    via = v5[:, :, 0:k][:, :, :, None].to_broadcast(
                        [128, t_n, k, k])
                    vjb = v5[:, :, 0:k][:, :, None, :].to_broadcast(
                        [128, t_n, k, k])
                    nc.vector.tensor_mul(vv[:, :, 0:k, 0:k], via, vjb)
                    svb = sv[:, :, None, None].to_broadcast([128, t_n, k, k])
                    nc.vector.tensor_mul(vv[:, :, 0:k, 0:k],
                                         vv[:, :, 0:k, 0:k], svb)
                    nc.vector.tensor_add(minv[:, :, 0:k, 0:k],
                                         minv[:, :, 0:k, 0:k],
                                         vv[:, :, 0:k, 0:k])
                    # Minv[0:k,k] = -sinv v ; mirror ; Minv[k,k] = sinv
                    svk = sv[:, :, None].to_broadcast([128, t_n, k])
                    nc.vector.scalar_tensor_tensor(
                        out=minv[:, :, 0:k, k], in0=v5[:, :, 0:k], scalar=-1.0,
                        in1=svk, op0=Alu.mult, op1=Alu.mult)
                    nc.gpsimd.tensor_copy(minv[:, :, k, 0:k],
                                          minv[:, :, 0:k, k])
                    nc.gpsimd.tensor_copy(minv[:, :, k, k], sv[:])
                    # c = Minv y
                    yb = y5[:, :, 0:k + 1][:, :, None, :].to_broadcast(
                        [128, t_n, k + 1, k + 1])
                    nc.vector.tensor_mul(vv[:, :, 0:k + 1, 0:k + 1],
                                         minv[:, :, 0:k + 1, 0:k + 1], yb)
                    nc.vector.tensor_reduce(out=cst[:, :, 0:k + 1],
                                            in_=vv[:, :, 0:k + 1, 0:k + 1],
                                            axis=X, op=Alu.add)

                # --- update mu = -A c (e-major) via PE transposes ---
                nc.gpsimd.tensor_scalar(negc[:, :, 0:k + 1],
                                        cst[:, :, 0:k + 1], -1.0,
                                        scalar2=None, op0=Alu.mult)
                for t in range(t_n):
                    prod = scr.tile([128, S, E], f32, tag="prod")
                    ncb = negc[:, t, 0:k + 1][:, :, None].to_broadcast(
                        [128, k + 1, E])
                    nc.vector.scalar_tensor_tensor(
                        out=prod[:, 0:k + 1, :], in0=a_st[:, t, 0:k + 1, :],
                        scalar=0.0, in1=ncb, op0=Alu.bypass, op1=Alu.mult)
                    pm = pmu.tile([E, 128], f32, tag="pm")
                    for j in range(k + 1):
                        nc.tensor.matmul(pm[:], prod[:, j, :], ident[:],
                                         start=(j == 0), stop=(j == k))
                    nc.scalar.activation(mu_em[:, t * 128:(t + 1) * 128],
                                         pm[:], Act.Copy)

            # ---------------- epilogue ----------------
            # res = s + mu = s - u ;  z_q = s - clip(res, -1, 1)
            res = st.tile([E, nsig], f32, tag="res")
            nc.vector.tensor_add(res[:], s_em[:], mu_em[:])
            # rec partial: sum over (e, n) of res^2 (scratch into mu_em)
            racc = st.tile([E, 1], f32, tag="racc")
            nc.scalar.activation(mu_em[:], res[:], Act.Square,
                                 accum_out=racc[:])
            nc.sync.dma_start(o_rec[:], racc[:])
            nc.vector.tensor_scalar(res[:], res[:], -1.0, scalar2=1.0,
                                    op0=Alu.max, op1=Alu.min)
            nc.vector.tensor_sub(mu_em[:], s_em[:], res[:])
            for b in range(bpc):
                eng = [nc.sync, nc.scalar][b % 2]
                eng.dma_start(
                    o_zq[b],
                    mu_em.rearrange("e (b h) -> e b h", b=bpc)[:, b, :])

            # l1 partial over coefficients
            cabs = st.tile([128, t_n, S], f32, tag="cabs")
            lacc = st.tile([128, 1], f32, tag="lacc")
            nc.scalar.activation(cabs[:], cst[:], Act.Abs, accum_out=lacc[:])
            nc.sync.dma_start(o_l1[:], lacc[:])

            # outputs: indices + coefficients (t, p, j)
            nc.sync.dma_start(o_idx.rearrange("t p j -> p t j"), idxf[:])
            nc.scalar.dma_start(o_c.rearrange("t p j -> p t j"), cst[:])

    nc.compile()
    return nc


def _get_nc(key="full"):
    if key not in _CACHE:
        _CACHE[key] = _build()
    return _CACHE[key]


def kernel(z, dictionary):
    import concourse.bass_utils as bass_utils

    z = np.ascontiguousarray(np.asarray(z, dtype=np.float32))
    d = np.ascontiguousarray(np.asarray(dictionary, dtype=np.float32))
    nc = _get_nc()

    dtr = np.ascontiguousarray(d.T)
    in_maps = []
    for c in range(N_CORES):
        zsl = np.ascontiguousarray(
            z[c * BPC:(c + 1) * BPC].reshape(BPC, E, HWF))
        in_maps.append({"zs": zsl, "dmat": d, "dtr": dtr})

    global LAST_EXEC_NS, LAST_RESULT
    if TRACE:
        _install_profhook()
    for attempt in range(2):
        res = bass_utils.run_bass_kernel_spmd(
            nc, in_maps, core_ids=list(range(N_CORES)), trace=TRACE,
            trace_cores=[0] if TRACE else None)
        ok = True
        for c in range(N_CORES):
            r = res.results[c]
            idx = r["o_idx"]
            if not np.isfinite(r["o_rec"]).all() or idx.min() < 0 \
                    or idx.max() >= K:
                ok = False
        if ok:
            break
    LAST_RESULT = res
    LAST_EXEC_NS = res.exec_time_ns

    zq = np.empty((B, E, 32, 32), np.float32)
    coeff = np.zeros((K, B * HWF), np.float32)
    rec_sum = 0.0
    l1_sum = 0.0
    nsig_all = B * HWF
    for c in range(N_CORES):
        r = res.results[c]
        zq[c * BPC:(c + 1) * BPC] = r["o_zq"].reshape(BPC, E, 32, 32)
        idx = r["o_idx"].reshape(NSIG, S).astype(np.int64)  # (t,p,j)->(n,j)
        cc = r["o_c"].reshape(NSIG, S)
        cc = np.clip(cc, -COEFF_CLAMP, COEFF_CLAMP)
        n0 = c * NSIG
        rows = np.arange(NSIG) + n0
        for j in range(S):
            coeff[idx[:, j], rows] = cc[:, j]
        rec_sum += float(r["o_rec"].sum())
        l1_sum += float(np.abs(cc).sum())

    rec_loss = rec_sum / (E * nsig_all)
    effective_cost = COMMITMENT_COST * (1.0 + S / 10.0)
    coeff_loss = l1_sum / (K * nsig_all)
    loss = np.float32(rec_loss + effective_cost * rec_loss + 0.001 * coeff_loss)
    return zq, loss, coeff


# revision 14
# speedup vs baseline: 1.1084x; 1.1084x over previous
"""Trainium2 Bass kernel for nn_DictionaryLearningBottleneck.

Computes batched OMP (5 iterations, 512-atom dictionary, 64-dim signals)
plus straight-through-quantized output and losses, data-parallel over the
signal axis across 8 NeuronCores.

Self-contained: hardcodes shapes; host-side prep is limited to layout
transforms (transposes/reshapes of the dictionary), output unsharding,
the final scalar loss reduction, and densifying the sparse coefficients.
"""
import numpy as np

B, E, HWF, K, S = 32, 64, 1024, 512, 5
N_CORES = 8
BPC = B // N_CORES            # batch slices per core
NSIG = BPC * HWF              # 4096 signals per core
T = NSIG // 128               # 32 tiles of 128 signals
NQ = 4                        # gather DMA queues (ucode MAX_SWDGE_QUEUES)
TQ = T // NQ                  # tiles per queue
EPS = 1e-10
NEGBIG = -1.0e30
COMMITMENT_COST = 0.25
COEFF_CLAMP = 1e7

_CACHE = {}
_OPS = {}
TRACE = False


def _register_custom_ops():
    """Register the fused selection ops into concourse's custom-DVE table."""
    if _OPS:
        return _OPS
    from concourse.dve_ops import (DveOp, OPS, _SUB_OPCODE_FOR_NAME,
                                   CUSTOM_DVE_SPECS)
    from concourse.dve_spec import (Spec, Src0, Src1, C0, C1, C2, One,
                                    lower, AluOp, Idx, _has_src1, select)
    from concourse.dve_uop import DveOpSpec

    FLT_MAX = np.float32(3.4028235e38)

    def ref_masksel(in0, in1, s0, s1, imm2):
        P, N = in0.shape[0], in0.shape[-1]
        pen = (in1.reshape(P, N).astype(np.float32) < imm2) * np.float32(imm2)
        body = (in0.reshape(P, N).astype(np.float32) - pen).astype(np.float32)
        acc = np.maximum(-FLT_MAX, body.max(axis=1, keepdims=True))
        return body, acc

    def ref_idxmin(in0, in1, s0, s1, imm2):
        P, N = in0.shape[0], in0.shape[-1]
        ar = np.arange(N, dtype=np.float32)[None, :]
        body = np.where(in0.reshape(P, N) >= s0, ar + 1.0,
                        np.float32(s1)).astype(np.float32)
        acc = np.minimum(np.float32(s1), body.min(axis=1, keepdims=True))
        return body, acc

    def make(name, spec):
        if name in _SUB_OPCODE_FOR_NAME:
            op = next(o for o in OPS if o.name == name)
            return op
        opcode = max(_SUB_OPCODE_FOR_NAME.values()) + 1
        _SUB_OPCODE_FOR_NAME[name] = opcode
        shas = {}
        for ver in ("v3", "v4"):
            u = lower(spec, ver=ver)
            shas[ver] = DveOpSpec(name=name, opcode=opcode, uops=u,
                                  rd1_en=_has_src1(spec)).sha(ver)
        op = DveOp(name, spec, False, shas)
        OPS.append(op)
        CUSTOM_DVE_SPECS[name] = spec
        return op

    _OPS["masksel"] = make("OMP_MASKSEL_ANT", Spec(
        body=Src0 - (Src1 < C2) * C2, accum=AluOp.MAX,
        reference=ref_masksel))
    _OPS["idxmin"] = make("OMP_IDXMIN_ANT", Spec(
        body=select(Src0 >= C0, Idx + One, C1),
        accum=AluOp.MIN, accum_init=C1,
        reference=ref_idxmin))
    return _OPS
LAST_EXEC_NS = None
LAST_RESULT = None


def _install_profhook():
    """Enable NTFF profiling through the axon tunnel (trace=True path)."""
    import sys, types
    if "antenv.axon_hooks" in sys.modules:
        return
    try:
        import antenv
        from trn_agent_boot.trn_boot import _ntff_profile_via_ctypes
        hook = _ntff_profile_via_ctypes("/opt/axon/libaxon_pjrt.so")
        mod = types.ModuleType("antenv.axon_hooks")
        mod.get_axon_ntff_profile_hook = lambda: hook
        mod.set_axon_ntff_profile_hook = lambda h: None
        sys.modules["antenv.axon_hooks"] = mod
        antenv.axon_hooks = mod
    except Exception:
        pass


def _build(nsig=NSIG, bpc=BPC, hwf=None):
    import concourse.tile as tile
    from concourse import bacc, mybir

    f32 = mybir.dt.float32
    i32 = mybir.dt.int32
    i16 = mybir.dt.int16
    Alu = mybir.AluOpType
    Act = mybir.ActivationFunctionType
    X = mybir.AxisListType.X

    t_n = nsig // 128
    tq = t_n // NQ
    if hwf is None:
        hwf = nsig // bpc

    ops = _register_custom_ops()

    nc = bacc.Bacc("TRN2", target_bir_lowering=False, debug=False,
                   num_devices=N_CORES, num_swdge_queues=NQ)

    zs = nc.dram_tensor("zs", [bpc, E, hwf], f32, kind="ExternalInput").ap()
    dmat = nc.dram_tensor("dmat", [E, K], f32, kind="ExternalInput").ap()
    dtr = nc.dram_tensor("dtr", [K, E], f32, kind="ExternalInput").ap()
    o_zq = nc.dram_tensor("o_zq", [bpc, E, hwf], f32, kind="ExternalOutput").ap()
    o_idx = nc.dram_tensor("o_idx", [t_n, 128, S], f32, kind="ExternalOutput").ap()
    o_c = nc.dram_tensor("o_c", [t_n, 128, S], f32, kind="ExternalOutput").ap()
    o_rec = nc.dram_tensor("o_rec", [E, 1], f32, kind="ExternalOutput").ap()
    o_l1 = nc.dram_tensor("o_l1", [128, 1], f32, kind="ExternalOutput").ap()

    with tile.TileContext(nc) as tc:
        with tc.tile_pool(name="st", bufs=1) as st, \
             tc.tile_pool(name="dram", bufs=1, space="DRAM") as dpool, \
             tc.tile_pool(name="pcorr", bufs=3, space="PSUM") as pcorr, \
             tc.tile_pool(name="pmu", bufs=3, space="PSUM") as pmu, \
             tc.tile_pool(name="scr", bufs=3) as scr, \
             tc.tile_pool(name="big", bufs=1) as big:

            # ---------------- static tiles ----------------
            s_em = st.tile([E, nsig], f32, tag="s_em")
            for b in range(bpc):
                eng = [nc.sync, nc.scalar][b % 2]
                eng.dma_start(
                    s_em.rearrange("e (b h) -> e b h", b=bpc)[:, b, :], zs[b])

            dD = st.tile([E, K], f32, tag="dD")
            nc.sync.dma_start(dD[:], dmat[:])

            iota_i = st.tile([128, K], i32, tag="iota_i")
            nc.gpsimd.iota(iota_i[:], pattern=[[1, K]], base=0,
                           channel_multiplier=0)
            iota_f = st.tile([128, K], f32, tag="iota_f")
            nc.vector.tensor_copy(iota_f[:], iota_i[:])

            iota_p = st.tile([128, 1], i32, tag="iota_p")
            nc.gpsimd.iota(iota_p[:], pattern=[[0, 1]], base=0,
                           channel_multiplier=1)
            iota_pf = st.tile([128, 1], f32, tag="iota_pf")
            nc.vector.tensor_copy(iota_pf[:], iota_p[:])
            ones128 = st.tile([128, 128], f32, tag="ones128")
            nc.vector.memset(ones128[:], 1.0)
            ident = st.tile([128, 128], f32, tag="ident")
            nc.vector.scalar_tensor_tensor(
                out=ident[:], in0=iota_f[:, 0:128], scalar=iota_pf[:],
                in1=ones128[:], op0=Alu.is_equal, op1=Alu.mult)

            plane = st.tile([128, t_n, K], mybir.dt.bfloat16, tag="plane")
            nc.vector.memset(plane[:], 3.0e38)
            a_st = st.tile([128, t_n, S, E], f32, tag="a_st")
            mu_em = st.tile([E, nsig], f32, tag="mu_em")
            nc.vector.memset(mu_em[:], 0.0)

            gmat = st.tile([128, t_n, S, S], f32, tag="gmat")
            minv = st.tile([128, t_n, S, S], f32, tag="minv")
            cst = st.tile([128, t_n, S], f32, tag="cst")
            nc.vector.memset(cst[:], 0.0)
            mst = st.tile([128, t_n, S], f32, tag="mst")
            c0a = st.tile([128, t_n, S], f32, tag="c0a")
            idxf = st.tile([128, t_n, S], f32, tag="idxf")
            y5 = st.tile([128, t_n, S], f32, tag="y5")
            v5 = st.tile([128, t_n, S], f32, tag="v5")
            tkk = st.tile([128, t_n], f32, tag="tkk")
            tss = st.tile([128, t_n], f32, tag="tss")
            sv = st.tile([128, t_n], f32, tag="sv")
            tk5 = st.tile([128, t_n, S], f32, tag="tk5")
            vv = st.tile([128, t_n, S, S], f32, tag="vv")
            negc = st.tile([128, t_n, S], f32, tag="negc")
            idx32 = st.tile([128, t_n], i32, tag="idx32")
            gidx16 = st.tile([128, t_n], i16, tag="gidx16")
            wrapa = st.tile([128, t_n * 8], i16, tag="wrapa")

            bufa = dpool.tile([16, t_n * 8], i16, tag="bufa")

            agq = [st.tile([128, tq, E], f32, name=f"agq{q}", tag=f"agq{q}")
                   for q in range(NQ)]

            # ---------------- OMP iterations ----------------
            for k in range(S):
                # --- selection phase (per tile) ---
                for t in range(t_n):
                    ps = pcorr.tile([128, K], f32, tag="ps")
                    nc.tensor.matmul(ps[:], s_em[:, t * 128:(t + 1) * 128],
                                     dD[:], start=True, stop=(k == 0))
                    if k > 0:
                        nc.tensor.matmul(ps[:], mu_em[:, t * 128:(t + 1) * 128],
                                         dD[:], start=False, stop=True)
                    corrm = scr.tile([128, K], f32, tag="corrm")
                    nc.vector._custom_dve(
                        ops["masksel"], out=corrm[:], in0=ps[:],
                        in1=plane[:, t, :], imm2=1.0e30,
                        accum_out=mst[:, t, k:k + 1])
                    plnew = scr.tile([128, K], mybir.dt.bfloat16, tag="plnew")
                    nc.vector._custom_dve(
                        ops["idxmin"], out=plnew[:], in0=corrm[:],
                        s0=mst[:, t, k:k + 1], s1=3.0e38,
                        accum_out=idxf[:, t, k:k + 1])
                    if k < S - 1:
                        nc.vector.tensor_tensor(
                            out=plane[:, t, :], in0=plane[:, t, :],
                            in1=plnew[:], op=Alu.min)

                # --- gather new atom rows ---
                nc.vector.tensor_scalar(idxf[:, :, k], idxf[:, :, k], 1.0,
                                        scalar2=None, op0=Alu.subtract)
                nc.vector.tensor_copy(idx32[:], idxf[:, :, k])
                nc.vector.tensor_copy(gidx16[:], idx32[:])
                nc.sync.dma_start(
                    bufa.rearrange("pl (t ph) -> ph pl t", ph=8), gidx16[:])
                for g in range(8):
                    eng = [nc.scalar, nc.sync][g % 2]
                    eng.dma_start(wrapa[g * 16:(g + 1) * 16, :], bufa[:])
                for q in range(NQ):
                    nc.gpsimd.dma_gather(
                        out_ap=agq[q][:], in_ap=dtr[:],
                        idxs_ap=wrapa[:, q * tq * 8:(q + 1) * tq * 8],
                        num_idxs=tq * 128, num_idxs_reg=tq * 128,
                        elem_size=E, queue_num=q)
                for q in range(NQ):
                    nc.scalar.activation(a_st[:, q * tq:(q + 1) * tq, k, :],
                                         agq[q][:], Act.Copy)

                # --- Gram column via dots: G[i,k] = <A_i, A_k>, i<=k ---
                th = t_n // 2
                for h in range(2):
                    hs = slice(h * th, (h + 1) * th)
                    ab = a_st[:, hs, k, :][:, :, None, :].to_broadcast(
                        [128, th, k + 1, E])
                    gp = big.tile([128, th, S, E], f32, tag="gp")
                    nc.vector.tensor_mul(gp[:, :, 0:k + 1, :],
                                         a_st[:, hs, 0:k + 1, :], ab)
                    nc.vector.tensor_reduce(out=gmat[:, hs, 0:k + 1, k],
                                            in_=gp[:, :, 0:k + 1, :],
                                            axis=X, op=Alu.add)
                if k > 0:
                    nc.gpsimd.tensor_copy(gmat[:, :, k, 0:k],
                                          gmat[:, :, 0:k, k])

                # --- solve (G + eps I) c = y via block-inverse recursion ---
                if k == 0:
                    nc.vector.tensor_copy(c0a[:, :, 0], mst[:, :, 0])
                    nc.vector.reciprocal(minv[:, :, 0, 0], gmat[:, :, 0, 0])
                    nc.vector.tensor_mul(cst[:, :, 0], minv[:, :, 0, 0],
                                         mst[:, :, 0])
                else:
                    # corr0 at the new atom: m_k + sum_{j<k} G[k,j] c_j
                    nc.gpsimd.tensor_mul(tk5[:, :, 0:k], gmat[:, :, k, 0:k],
                                         cst[:, :, 0:k])
                    nc.vector.tensor_reduce(out=tkk[:], in_=tk5[:, :, 0:k],
                                            axis=X, op=Alu.add)
                    nc.gpsimd.tensor_add(c0a[:, :, k], mst[:, :, k], tkk[:])
                    # y_i = corr0a_i - sum_{j<k} G[i,j] c_j   (i <= k)
                    cb = cst[:, :, 0:k][:, :, None, :].to_broadcast(
                        [128, t_n, k + 1, k])
                    nc.vector.tensor_mul(vv[:, :, 0:k + 1, 0:k],
                                         gmat[:, :, 0:k + 1, 0:k], cb)
                    nc.vector.tensor_reduce(out=y5[:, :, 0:k + 1],
                                            in_=vv[:, :, 0:k + 1, 0:k],
                                            axis=X, op=Alu.add)
                    nc.vector.tensor_sub(y5[:, :, 0:k + 1],
                                         c0a[:, :, 0:k + 1],
                                         y5[:, :, 0:k + 1])
                    # v = Minv b ; ss = G[k,k] - b.v ; sinv = 1/ss
                    bb = gmat[:, :, 0:k, k][:, :, None, :].to_broadcast(
                        [128, t_n, k, k])
                    nc.vector.tensor_mul(vv[:, :, 0:k, 0:k],
                                         minv[:, :, 0:k, 0:k], bb)
                    nc.vector.tensor_reduce(out=v5[:, :, 0:k],
                                            in_=vv[:, :, 0:k, 0:k],
                                            axis=X, op=Alu.add)
                    nc.gpsimd.tensor_mul(tk5[:, :, 0:k], gmat[:, :, 0:k, k],
                                         v5[:, :, 0:k])
                    nc.vector.tensor_reduce(out=tkk[:], in_=tk5[:, :, 0:k],
                                            axis=X, op=Alu.add)
                    nc.gpsimd.tensor_sub(tss[:], gmat[:, :, k, k], tkk[:])
                    nc.vector.reciprocal(sv[:], tss[:])
                    # Minv[0:k,0:k] += sinv * v v^T
                    via = v5[:, :, 0:k][:, :, :, None].to_broadcast(
                        [128, t_n, k, k])
                    vjb = v5[:, :, 0:k][:, :, None, :].to_broadcast(
                        [128, t_n, k, k])
                    nc.vector.tensor_mul(vv[:, :, 0:k, 0:k], via, vjb)
                    svb = sv[:, :, None, None].to_broadcast([128, t_n, k, k])
                    nc.vector.tensor_mul(vv[:, :, 0:k, 0:k],
                                         vv[:, :, 0:k, 0:k], svb)
                    nc.vector.tensor_add(minv[:, :, 0:k, 0:k],
                                         minv[:, :, 0:k, 0:k],
                                         vv[:, :, 0:k, 0:k])
                    # Minv[0:k,k] = -sinv v ; mirror ; Minv[k,k] = sinv
                    svk = sv[:, :, None].to_broadcast([128, t_n, k])
                    nc.vector.scalar_tensor_tensor(
                        out=minv[:, :, 0:k, k], in0=v5[:, :, 0:k], scalar=-1.0,
                        in1=svk, op0=Alu.mult, op1=Alu.mult)
                    nc.gpsimd.tensor_copy(minv[:, :, k, 0:k],
                                          minv[:, :, 0:k, k])
                    nc.gpsimd.tensor_copy(minv[:, :, k, k], sv[:])
                    # c = Minv y
                    yb = y5[:, :, 0:k + 1][:, :, None, :].to_broadcast(
                        [128, t_n, k + 1, k + 1])
                    nc.vector.tensor_mul(vv[:, :, 0:k + 1, 0:k + 1],
                                         minv[:, :, 0:k + 1, 0:k + 1], yb)
                    nc.vector.tensor_reduce(out=cst[:, :, 0:k + 1],
                                            in_=vv[:, :, 0:k + 1, 0:k + 1],
                                            axis=X, op=Alu.add)

                # --- update mu = -A c (e-major) via PE transposes ---
                nc.gpsimd.tensor_scalar(negc[:, :, 0:k + 1],
                                        cst[:, :, 0:k + 1], -1.0,
                                        scalar2=None, op0=Alu.mult)
                for t in range(t_n):
                    prod = scr.tile([128, S, E], f32, tag="prod")
                    ncb = negc[:, t, 0:k + 1][:, :, None].to_broadcast(
                        [128, k + 1, E])
                    nc.vector.scalar_tensor_tensor(
                        out=prod[:, 0:k + 1, :], in0=a_st[:, t, 0:k + 1, :],
                        scalar=0.0, in1=ncb, op0=Alu.bypass, op1=Alu.mult)
                    pm = pmu.tile([E, 128], f32, tag="pm")
                    for j in range(k + 1):
                        nc.tensor.matmul(pm[:], prod[:, j, :], ident[:],
                                         start=(j == 0), stop=(j == k))
                    nc.scalar.activation(mu_em[:, t * 128:(t + 1) * 128],
                                         pm[:], Act.Copy)

            # ---------------- epilogue ----------------
            # res = s + mu = s - u ;  z_q = s - clip(res, -1, 1)
            res = st.tile([E, nsig], f32, tag="res")
            nc.vector.tensor_add(res[:], s_em[:], mu_em[:])
            # rec partial: sum over (e, n) of res^2 (scratch into mu_em)
            racc = st.tile([E, 1], f32, tag="racc")
            nc.scalar.activation(mu_em[:], res[:], Act.Square,
                                 accum_out=racc[:])
            nc.sync.dma_start(o_rec[:], racc[:])
            nc.vector.tensor_scalar(res[:], res[:], -1.0, scalar2=1.0,
                                    op0=Alu.max, op1=Alu.min)
            nc.vector.tensor_sub(mu_em[:], s_em[:], res[:])
            for b in range(bpc):
                eng = [nc.sync, nc.scalar][b % 2]
                eng.dma_start(
                    o_zq[b],
                    mu_em.rearrange("e (b h) -> e b h", b=bpc)[:, b, :])

            # l1 partial over coefficients
            cabs = st.tile([128, t_n, S], f32, tag="cabs")
            lacc = st.tile([128, 1], f32, tag="lacc")
            nc.scalar.activation(cabs[:], cst[:], Act.Abs, accum_out=lacc[:])
            nc.sync.dma_start(o_l1[:], lacc[:])

            # outputs: indices + coefficients (t, p, j)
            nc.sync.dma_start(o_idx.rearrange("t p j -> p t j"), idxf[:])
            nc.scalar.dma_start(o_c.rearrange("t p j -> p t j"), cst[:])

    nc.compile()
    return nc


def _get_nc(key="full"):
    if key not in _CACHE:
        _CACHE[key] = _build()
    return _CACHE[key]


def kernel(z, dictionary):
    import concourse.bass_utils as bass_utils

    z = np.ascontiguousarray(np.asarray(z, dtype=np.float32))
    d = np.ascontiguousarray(np.asarray(dictionary, dtype=np.float32))
    nc = _get_nc()

    dtr = np.ascontiguousarray(d.T)
    in_maps = []
    for c in range(N_CORES):
        zsl = np.ascontiguousarray(
            z[c * BPC:(c + 1) * BPC].reshape(BPC, E, HWF))
        in_maps.append({"zs": zsl, "dmat": d, "dtr": dtr})

    global LAST_EXEC_NS, LAST_RESULT
    if TRACE:
        _install_profhook()
    for attempt in range(2):
        res = bass_utils.run_bass_kernel_spmd(
            nc, in_maps, core_ids=list(range(N_CORES)), trace=TRACE,
            trace_cores=[0] if TRACE else None)
        ok = True
        for c in range(N_CORES):
            r = res.results[c]
            idx = r["o_idx"]
            if not np.isfinite(r["o_rec"]).all() or idx.min() < 0 \
                    or idx.max() >= K:
                ok = False
        if ok:
            break
    LAST_RESULT = res
    LAST_EXEC_NS = res.exec_time_ns

    zq = np.empty((B, E, 32, 32), np.float32)
    coeff = np.zeros((K, B * HWF), np.float32)
    rec_sum = 0.0
    l1_sum = 0.0
    nsig_all = B * HWF
    for c in range(N_CORES):
        r = res.results[c]
        zq[c * BPC:(c + 1) * BPC] = r["o_zq"].reshape(BPC, E, 32, 32)
        idx = r["o_idx"].reshape(NSIG, S).astype(np.int64)  # (t,p,j)->(n,j)
        cc = r["o_c"].reshape(NSIG, S)
        cc = np.clip(cc, -COEFF_CLAMP, COEFF_CLAMP)
        n0 = c * NSIG
        rows = np.arange(NSIG) + n0
        for j in range(S):
            coeff[idx[:, j], rows] = cc[:, j]
        rec_sum += float(r["o_rec"].sum())
        l1_sum += float(np.abs(cc).sum())

    rec_loss = rec_sum / (E * nsig_all)
    effective_cost = COMMITMENT_COST * (1.0 + S / 10.0)
    coeff_loss = l1_sum / (K * nsig_all)
    loss = np.float32(rec_loss + effective_cost * rec_loss + 0.001 * coeff_loss)
    return zq, loss, coeff
